# revision 40
# baseline (speedup 1.0000x reference)
"""Causal multi-head self-attention with RoPE on 8 Trainium2 NeuronCores.

Full-input contract: kernel(**inputs) takes the complete tensors and returns
the complete [B, S, D] output. Internally shards (batch x head-group) across
8 cores: core c handles batch c//2 and heads (c%2)*8 .. (c%2)*8+8. Each core
computes its 8 heads' attention and a partial output projection; a pairwise
AllReduce (cores 2b, 2b+1) completes the projection sum.

All matmul operands are bf16 (cast host-side); accumulation stays fp32 in
PSUM. Attention value-product is q-partition-major ("flipped PV"): per
(head, 128-query tile) the psum is [q=128, 65] with column 64 accumulating
the softmax denominator, so normalization is a single per-partition divide.
Scores/exp/PV use exact-causal variable widths. RoPE even/odd pairs are
separated via a host-side permutation of the q/k weight rows.
"""

import numpy as np

import concourse.bass as bass
import concourse.mybir as mybir
import concourse.tile as tile
from concourse import bacc
from concourse.bass_utils import run_bass_kernel_spmd

F32 = mybir.dt.float32
BF16 = mybir.dt.bfloat16
AF = mybir.ActivationFunctionType
ALU = mybir.AluOpType

P = 128          # partitions
SQ = 512         # q-chunk
DK = 64          # head dim
NH = 8           # heads per core
DLOC = NH * DK   # 512 local out-features for q/k/v
THETA = 10000.0

B, S, D, H = 4, 2048, 1024, 16
N_CORES = 8

KC = D // P      # 8 contraction chunks
SJ = S // SQ     # 4 q-chunks
STJ = SQ // P    # 4 s-tiles per q-chunk
ST = S // P      # 16 s-tiles total


def build_attention_program(DIN=D, DOUT=D, SEQ=S, all_reduce=True, groups=None, reps=1,
                            debug=False):
    """One SPMD Bass program. Per-core DRAM I/O:
      xt    [DIN, SEQ]   bf16  x[b].T
      wqt   [DIN, DLOC]  bf16  wq rows (perm: per-half E-block/O-block) transposed
      wkt   [DIN, DLOC]  bf16  likewise
      wvt   [DIN, DLOC]  bf16  wv rows (natural) transposed
      wot   [DLOC, DOUT] bf16  wo cols (natural) transposed
      cos4  [P, SEQ]     f32   cos table, 4x stacked [32, SEQ]
      sin4  [P, SEQ]     f32
      tri   [P, P]       bf16  upper-tri keep mask (c >= r)
      ident [P, P]       bf16  identity for PE transpose
      outp  [SJ, DOUT, SQ] f32 out^T, j-chunked
    """
    nc = bacc.Bacc(
        "TRN2",
        target_bir_lowering=False,
        debug=False,
        num_devices=(len(groups) * len(groups[0]) if groups else 1),
    )
    xt = nc.declare_dram_parameter("xt", [DIN, SEQ], BF16, isOutput=False)
    wqt = nc.declare_dram_parameter("wqt", [DIN, DLOC], BF16, isOutput=False)
    wkt = nc.declare_dram_parameter("wkt", [DIN, DLOC], BF16, isOutput=False)
    wvt = nc.declare_dram_parameter("wvt", [DIN, DLOC], BF16, isOutput=False)
    wot = nc.declare_dram_parameter("wot", [DLOC, DOUT], BF16, isOutput=False)
    cos4 = nc.declare_dram_parameter("cos4", [P, SEQ], F32, isOutput=False)
    sin4 = nc.declare_dram_parameter("sin4", [P, SEQ], F32, isOutput=False)
    tri = nc.declare_dram_parameter("tri", [P, P], BF16, isOutput=False)
    ident = nc.declare_dram_parameter("ident", [P, P], BF16, isOutput=False)
    outp = nc.declare_dram_parameter("outp", [SJ, DOUT, SQ], F32, isOutput=True)
    if debug:
        dbg_qE = nc.declare_dram_parameter("dbg_qE", [2, P, SQ], BF16, isOutput=True)
        dbg_kE = nc.declare_dram_parameter("dbg_kE", [2, P, SEQ], BF16, isOutput=True)
        dbg_qO = nc.declare_dram_parameter("dbg_qO", [2, P, SQ], BF16, isOutput=True)
        dbg_kO = nc.declare_dram_parameter("dbg_kO", [2, P, SEQ], BF16, isOutput=True)
        dbg_v = nc.declare_dram_parameter("dbg_v", [P, ST, NH, DK + 1], BF16, isOutput=True)
        dbg_ot = nc.declare_dram_parameter("dbg_ot", [P, NH * STJ * DK], BF16, isOutput=True)
        dbg_otT = nc.declare_dram_parameter("dbg_otT", [P, STJ, SQ], BF16, isOutput=True)

    from contextlib import ExitStack

    # PV psum packing: slot (head-in-half, qtile) -> (bank, idx), 6 per bank
    def pv_slice(ps, hd, qt):
        slot = hd * STJ + qt
        b, i = slot // 6, slot % 6
        return ps[:, b, i * (DK + 1) : (i + 1) * (DK + 1)]

    with tile.TileContext(nc) as tc, ExitStack() as ctx:
        ctx.enter_context(nc.allow_low_precision(reason="bf16 matmuls"))
        consts = ctx.enter_context(tc.tile_pool(name="consts", bufs=1))
        tabs = ctx.enter_context(tc.tile_pool(name="tabs", bufs=1))
        xload = ctx.enter_context(tc.tile_pool(name="xload", bufs=2))
        qk_pool = ctx.enter_context(tc.tile_pool(name="qk", bufs=1))
        v_pool = ctx.enter_context(tc.tile_pool(name="vp", bufs=1))
        ot_pool = ctx.enter_context(tc.tile_pool(name="ot", bufs=1))
        otT_pool = ctx.enter_context(tc.tile_pool(name="otT", bufs=1))
        tmp_pool = ctx.enter_context(tc.tile_pool(name="tmp", bufs=2))
        pt_pool = ctx.enter_context(tc.tile_pool(name="pt", bufs=32))
        ob_pool = ctx.enter_context(tc.tile_pool(name="ob", bufs=2))
        dram_pool = ctx.enter_context(tc.tile_pool(name="dram", bufs=2, space="DRAM"))
        psS = ctx.enter_context(tc.tile_pool(name="psS", bufs=2, space="PSUM"))
        psV = ctx.enter_context(tc.tile_pool(name="psV", bufs=1, space="PSUM"))
        for _rep in range(reps):
            tri_sb = consts.tile([P, P], BF16, tag="tri")
            nc.sync.dma_start(tri_sb[:], tri[:])
            id_sb = consts.tile([P, P], BF16, tag="ident")
            nc.sync.dma_start(id_sb[:], ident[:])
            wq_sb = consts.tile([P, KC, DLOC], BF16, tag="wq")
            nc.sync.dma_start(wq_sb[:], wqt.rearrange("(k p) c -> p k c", p=P))
            wk_sb = consts.tile([P, KC, DLOC], BF16, tag="wk")
            nc.sync.dma_start(wk_sb[:], wkt.rearrange("(k p) c -> p k c", p=P))
            wv_sb = consts.tile([P, KC, DLOC], BF16, tag="wv")
            nc.sync.dma_start(wv_sb[:], wvt.rearrange("(k p) c -> p k c", p=P))
            wo_sb = consts.tile([P, DLOC // P, DOUT], BF16, tag="wo")
            nc.sync.dma_start(wo_sb[:], wot.rearrange("(k p) c -> p k c", p=P))
            cos_sb = tabs.tile([P, SEQ], F32, tag="cos")
            nc.sync.dma_start(cos_sb[:], cos4[:])
            sin_sb = tabs.tile([P, SEQ], F32, tag="sin")
            nc.sync.dma_start(sin_sb[:], sin4[:])

            # rotated k in [dout, s] layout, per half: E block (4 heads x 32
            # even dims) and O block (odd dims). q is per-j (ring 2).
            kE = [qk_pool.tile([P, SEQ], BF16, tag=f"kE{g}", name=f"kE{g}") for g in range(2)]
            kO = [qk_pool.tile([P, SEQ], BF16, tag=f"kO{g}", name=f"kO{g}") for g in range(2)]
            # v natural [s, dv]: per s-tile, per head: 64 dims + ones col
            v_sb = v_pool.tile([P, ST, NH, DK + 1], BF16, tag="v")
            nc.vector.memset(v_sb[:, :, :, DK : DK + 1], 1.0)
            # attention out per j, head-major flat: col (hd*4+qt)*64+d
            ot_sb = ot_pool.tile([P, NH * STJ * DK], BF16, tag="ot")
            otr = ot_sb.rearrange("p (h q d) -> p h q d", q=STJ, d=DK)
            otT_sb = otT_pool.tile([P, STJ, SQ], BF16, tag="otT")

            def qkv_emitters(j):
                """Projections + RoPE for chunk j as three PE chunks (q, k, v)
                using the psV 'pv' psum tag (idle during Phase A), so they can
                zip into Phase A without touching the scores 'sc' ring.
                Returns (emitters, (qE, qO))."""
                js = slice(j * SQ, (j + 1) * SQ)
                xt_sb = xload.tile([P, KC, SQ], BF16, tag="xt", name=f"xt{j}")
                nc.sync.dma_start(
                    xt_sb[:], xt[:, js].rearrange("(k p) s -> p k s", p=P)
                )
                qE = [qk_pool.tile([P, SQ], BF16, tag=f"qE{g}", name=f"qE{g}_{j}",
                                   bufs=2) for g in range(2)]
                qO = [qk_pool.tile([P, SQ], BF16, tag=f"qO{g}", name=f"qO{g}_{j}",
                                   bufs=2) for g in range(2)]

                def em_qk(tname, wsb):
                    # 4 chains (g x E/O), one per psum bank
                    ps = psV.tile([P, 4, SQ], F32, tag="pv", name=f"ps_{tname}{j}")
                    for g in range(2):
                        for eo in range(2):
                            c0 = g * 256 + eo * P
                            for kk in range(KC):
                                nc.tensor.matmul(
                                    ps[:, 2 * g + eo, :],
                                    lhsT=(wsb[:, kk, c0 : c0 + P]),
                                    rhs=(xt_sb[:, kk, :]),
                                    start=(kk == 0),
                                    stop=(kk == KC - 1),
                                )
                    # RoPE: yE = cos*E - sin*O ; yO = sin*E + cos*O
                    # psum-reading mults on DVE; SBUF-only combines on Pool
                    for g in range(2):
                        if tname == "q":
                            dE, dO = qE[g][:], qO[g][:]
                        else:
                            dE, dO = kE[g][:, js], kO[g][:, js]
                        psE, psO = ps[:, 2 * g, :], ps[:, 2 * g + 1, :]
                        t1 = tmp_pool.tile([P, SQ], F32, tag="t1")
                        nc.vector.tensor_tensor(t1[:], cos_sb[:, js], psE, ALU.mult)
                        t2 = tmp_pool.tile([P, SQ], F32, tag="t2")
                        nc.vector.tensor_tensor(t2[:], sin_sb[:, js], psO, ALU.mult)
                        nc.gpsimd.tensor_tensor(dE, t1[:], t2[:], ALU.subtract)
                        t3 = tmp_pool.tile([P, SQ], F32, tag="t1")
                        nc.vector.tensor_tensor(t3[:], sin_sb[:, js], psE, ALU.mult)
                        t4 = tmp_pool.tile([P, SQ], F32, tag="t2")
                        nc.vector.tensor_tensor(t4[:], cos_sb[:, js], psO, ALU.mult)
                        nc.gpsimd.tensor_tensor(dO, t3[:], t4[:], ALU.add)

                def em_v():
                    # v projection: 4 s-tile chains, one per bank
                    ps = psV.tile([P, 4, SQ], F32, tag="pv", name=f"ps_v{j}")
                    for i in range(STJ):
                        for kk in range(KC):
                            nc.tensor.matmul(
                                ps[:, i, :],
                                lhsT=(xt_sb[:, kk, i * P : (i + 1) * P]),
                                rhs=(wv_sb[:, kk, :]),
                                start=(kk == 0),
                                stop=(kk == KC - 1),
                            )
                    nc.vector.tensor_copy(
                        v_sb[:, j * STJ : (j + 1) * STJ, :, 0:DK],
                        ps.rearrange("p i (h d) -> p i h d", h=NH),
                    )

                ems = [
                    lambda: em_qk("q", wq_sb),
                    lambda: em_qk("k", wk_sb),
                    em_v,
                ]
                return ems, (qE, qO)

            def emit_scores(j, half, qE, qO, zip_work=None):
                """Phase A: scores + exp + mask for all causal tiles of
                (half, j). Returns pt tiles keyed (t, pair). Phase A is
                Act-limited; `zip_work` (a list of emitter callables of
                PE-side work with ready deps) is drained evenly between
                tiles to fill the PE gaps."""
                ntile = (j + 1) * STJ
                zw = list(zip_work or [])
                pts = {}
                for t in range(ntile):
                    ts_ = slice(t * P, (t + 1) * P)
                    tq = t - j * STJ  # >=0 on diagonal wedge
                    start_c = max(tq, 0) * P
                    for p in range(2):  # head pairs in this half
                        ssc = psS.tile([P, 2, SQ], F32, tag="sc", name=f"ssc{p}")
                        for hh in range(2):
                            hd = 2 * p + hh
                            bp = hd * 32
                            nc.tensor.matmul(
                                ssc[:, hh, start_c:SQ],
                                lhsT=(kE[half][bp : bp + 32, ts_]),
                                rhs=(qE[half][bp : bp + 32, start_c:SQ]),
                                start=True,
                                stop=False,
                                tile_position=(bp, 0),
                            )
                            nc.tensor.matmul(
                                ssc[:, hh, start_c:SQ],
                                lhsT=(kO[half][bp : bp + 32, ts_]),
                                rhs=(qO[half][bp : bp + 32, start_c:SQ]),
                                start=False,
                                stop=True,
                                tile_position=(bp, 0),
                            )
                        pt = pt_pool.tile([P, 2, SQ], BF16, tag="pt",
                                          name=f"pt{t}_{p}")
                        nc.scalar.activation(
                            pt[:, :, start_c:SQ], ssc[:, :, start_c:SQ],
                            AF.Exp, scale=0.125,
                        )
                        if tq >= 0:
                            # diagonal block: zero where q < sk
                            nc.vector.tensor_tensor(
                                pt[:, :, start_c : start_c + P],
                                pt[:, :, start_c : start_c + P],
                                tri_sb[:, None, :].broadcast_to((P, 2, P)),
                                ALU.mult,
                            )
                        pts[(t, p)] = pt
                    # drain a proportional share of the zipped PE work
                    nchunk = max(1, len(zw) // (ntile - t)) if zw else 0
                    for _ in range(nchunk):
                        if zw:
                            zw.pop(0)()
                for em in zw:
                    em()
                return pts

            def pv_emitters(j, half, pts):
                """Phase B: PV chains, qt-outer; one psum bank per head
                (a start=True resets its whole 2KB bank per partition lane,
                so banks hold exactly one live accumulation chain). Batched
                norm per qt. Returned as emitter chunks for zipping."""
                pv_box = []

                def em_chain(qt, hd):
                    if not pv_box:
                        pv_box.append(
                            psV.tile([P, STJ, SQ], F32, tag="pv", name=f"pv{half}")
                        )
                    pv = pv_box[0]
                    last = j * STJ + qt
                    for t in range(last + 1):
                        nc.tensor.matmul(
                            pv[:, hd, 0 : DK + 1],
                            lhsT=(pts[(t, hd // 2)][:, hd % 2, qt * P : (qt + 1) * P]),
                            rhs=(v_sb[:, t, half * 4 + hd, :]),
                            start=(t == 0),
                            stop=(t == last),
                        )

                def em_norm(qt):
                    pv = pv_box[0]
                    rden = tmp_pool.tile([P, 4], F32, tag="rden")
                    nc.vector.reciprocal(rden[:], pv[:, :, DK])
                    nc.vector.tensor_tensor(
                        otr[:, half * 4 : half * 4 + 4, qt, :],
                        pv[:, :, 0:DK],
                        rden[:, :, None].broadcast_to((P, 4, DK)),
                        ALU.mult,
                    )

                ems = []
                for qt in range(STJ):
                    for hd in range(4):
                        ems.append(lambda qt=qt, hd=hd: em_chain(qt, hd))
                    ems.append(lambda qt=qt: em_norm(qt))
                return ems

            def emit_transpose(j, qt):
                """ot [q, dloc] -> otT [dloc, q] for one qtile: an 8-matmul
                accumulation chain (regular matmul vs identity -> f32 psum,
                eight 512B outputs disjoint within one 2KB bank)."""
                tp = psS.tile([P, STJ, P], F32, tag="sc", name=f"tp{qt}")
                for ic in range(STJ):
                    for u in range(2):
                        hd = 2 * ic + u
                        c0 = (hd * STJ + qt) * DK
                        # per-u chains: psum has_written bits are per
                        # partition lane, so each 64-partition strip
                        # needs its own start to clear stale state
                        nc.tensor.matmul(
                            tp[u * DK : (u + 1) * DK, ic, :],
                            lhsT=ot_sb[:, c0 : c0 + DK],
                            rhs=id_sb[:],
                            start=(ic == 0),
                            stop=(ic == STJ - 1),
                        )
                if qt % 2 == 0:
                    nc.scalar.copy(otT_sb[:, :, qt * P : (qt + 1) * P], tp[:])
                else:
                    nc.vector.tensor_copy(
                        otT_sb[:, :, qt * P : (qt + 1) * P], tp[:]
                    )

            def transp_oproj_emitters(j):
                """Transposes, output projection, and AllReduce for chunk j,
                as emitter chunks (zipped into the next j's Phase A)."""
                ems = [lambda qt=qt: emit_transpose(j, qt) for qt in range(STJ)]
                box = []

                def em_oproj(sblk):
                    if not box:
                        box.append(dram_pool.tile([DOUT, SQ], F32, tag="opart",
                                                  name=f"op_dram{j}"))
                    op_dram = box[0]
                    ps = psS.tile([P, 2, SQ], F32, tag="sc", name=f"op{sblk}")
                    for u in range(2):
                        dc = 2 * sblk + u
                        for ic in range(STJ):
                            nc.tensor.matmul(
                                ps[:, u, :],
                                lhsT=(wo_sb[:, ic, dc * P : (dc + 1) * P]),
                                rhs=(otT_sb[:, ic, :]),
                                start=(ic == 0),
                                stop=(ic == STJ - 1),
                            )
                    ob = ob_pool.tile([P, 2, SQ], F32, tag="ob")
                    nc.vector.tensor_copy(ob[:], ps[:])
                    for u in range(2):
                        dc = 2 * sblk + u
                        nc.sync.dma_start(
                            op_dram[dc * P : (dc + 1) * P, :], ob[:, u, :]
                        )

                def em_ar():
                    op_dram = box[0]
                    if all_reduce:
                        ar_dram = dram_pool.tile([DOUT, SQ], F32, tag="arout",
                                                 name=f"ar_dram{j}")
                        nc.gpsimd.collective_compute(
                            "AllReduce",
                            ALU.add,
                            replica_groups=groups,
                            ins=[op_dram.opt()],
                            outs=[ar_dram.opt()],
                        )
                        nc.sync.dma_start(outp[j], ar_dram[:])
                    else:
                        nc.sync.dma_start(outp[j], op_dram[:])

                ems += [lambda s=sblk: em_oproj(s) for sblk in range(DOUT // P // 2)]
                ems.append(em_ar)
                return ems

            qkv_ems, qn = qkv_emitters(0)
            for em in qkv_ems:
                em()
            for j in range(SJ):
                pts0 = emit_scores(j, 0, *qn)
                for em in pv_emitters(j, 0, pts0):
                    em()
                if j + 1 < SJ:
                    qkv_ems, qn_next = qkv_emitters(j + 1)
                else:
                    qkv_ems = []
                pts1 = emit_scores(j, 1, *qn, zip_work=qkv_ems)
                for em in pv_emitters(j, 1, pts1):
                    em()
                for em in transp_oproj_emitters(j):
                    em()
                if j + 1 < SJ:
                    qn = qn_next
            if debug:
                nc.sync.dma_start(dbg_ot[:], ot_sb[:])
                nc.sync.dma_start(dbg_otT[:], otT_sb[:])

            if debug:
                for g in range(2):
                    nc.sync.dma_start(dbg_qE[g], qn[0][g][:])
                    nc.sync.dma_start(dbg_kE[g], kE[g][:])
                    nc.sync.dma_start(dbg_qO[g], qn[1][g][:])
                    nc.sync.dma_start(dbg_kO[g], kO[g][:])
                nc.sync.dma_start(dbg_v[:], v_sb[:])

    nc.finalize()
    return nc


def make_perms():
    """perm (q/k): per half, E-block then O-block across the half's 4 heads."""
    perm = []
    for half in range(2):
        for par in range(2):  # 0=E, 1=O
            for h in range(4 * half, 4 * half + 4):
                for i in range(32):
                    perm.append(h * DK + 2 * i + par)
    return np.array(perm)


def make_tables(token_positions, SEQ):
    pos = np.asarray(token_positions).astype(np.float32)
    inv_freq = (1.0 / (THETA ** (np.arange(0, DK, 2, dtype=np.float32) / DK))).astype(
        np.float32
    )
    freqs = pos[:, None] * inv_freq[None, :]  # [S, 32]
    cosT = np.cos(freqs).T.astype(np.float32)  # [32, S]
    sinT = np.sin(freqs).T.astype(np.float32)
    return (
        np.ascontiguousarray(np.tile(cosT, (4, 1))),
        np.ascontiguousarray(np.tile(sinT, (4, 1))),
    )


def shard_inputs(x, token_positions, wq, wk, wv, wo):
    """Build the 8 per-core input maps."""
    bf16 = mybir.dt.np(BF16)
    perm = make_perms()
    cos4, sin4 = make_tables(token_positions, x.shape[1])
    tri = np.triu(np.ones((P, P), dtype=np.float32)).astype(bf16)
    ident = np.eye(P, dtype=np.float32).astype(bf16)
    in_maps = []
    for c in range(N_CORES):
        b, hg = c // 2, c % 2
        rows = hg * DLOC
        gperm = perm + rows
        rsl = slice(rows, rows + DLOC)
        in_maps.append(
            {
                "xt": np.ascontiguousarray(x[b].T).astype(bf16),
                "wqt": np.ascontiguousarray(wq[gperm, :].T.astype(bf16)),
                "wkt": np.ascontiguousarray(wk[gperm, :].T.astype(bf16)),
                "wvt": np.ascontiguousarray(wv[rsl, :].T.astype(bf16)),
                "wot": np.ascontiguousarray(wo[:, rsl].T.astype(bf16)),
                "cos4": cos4,
                "sin4": sin4,
                "tri": tri,
                "ident": ident,
            }
        )
    return in_maps


def gather_output(res):
    out = np.empty((B, S, D), dtype=np.float32)
    for b in range(B):
        chunks = res.results[2 * b]["outp"]  # [SJ, D, SQ]
        outT = np.concatenate(list(chunks), axis=1)  # [D, S]
        out[b] = outT.T
    return out


_NC_CACHE = {}


def kernel(x, token_positions, wq, wk, wv, wo, trace=False):
    x = np.asarray(x, dtype=np.float32)
    wq = np.asarray(wq, dtype=np.float32)
    wk = np.asarray(wk, dtype=np.float32)
    wv = np.asarray(wv, dtype=np.float32)
    wo = np.asarray(wo, dtype=np.float32)

    key = "full"
    if key not in _NC_CACHE:
        _NC_CACHE[key] = build_attention_program(
            DIN=D,
            DOUT=D,
            SEQ=S,
            all_reduce=True,
            groups=[[0, 1], [2, 3], [4, 5], [6, 7]],
        )
    nc = _NC_CACHE[key]

    in_maps = shard_inputs(x, token_positions, wq, wk, wv, wo)
    res = run_bass_kernel_spmd(nc, in_maps, list(range(N_CORES)), trace=trace)
    out = gather_output(res)
    if trace:
        return out, res
    return out


# revision 46
# speedup vs baseline: 1.4879x; 1.4879x over previous
"""Causal multi-head self-attention with RoPE on 8 Trainium2 NeuronCores.

Full-input contract: kernel(**inputs) takes the complete tensors and returns
the complete [B, S, D] output. Internally shards (batch x head-group) across
8 cores: core c handles batch c//2 and heads (c%2)*8 .. (c%2)*8+8. Each core
computes its 8 heads' attention and a partial output projection; a pairwise
AllReduce (cores 2b, 2b+1) completes the projection sum.

All matmul operands are bf16 (cast host-side); accumulation stays fp32 in
PSUM. Attention value-product is q-partition-major ("flipped PV"): per
(head, 128-query tile) the psum is [q=128, 65] with column 64 accumulating
the softmax denominator, so normalization is a single per-partition divide.
Scores/exp/PV use exact-causal variable widths. RoPE even/odd pairs are
separated via a host-side permutation of the q/k weight rows.
"""

import numpy as np

import concourse.bass as bass
import concourse.mybir as mybir
import concourse.tile as tile
from concourse import bacc
from concourse.bass_utils import run_bass_kernel_spmd

F32 = mybir.dt.float32
BF16 = mybir.dt.bfloat16
AF = mybir.ActivationFunctionType
ALU = mybir.AluOpType

P = 128          # partitions
SQ = 512         # q-chunk
DK = 64          # head dim
NH = 8           # heads per core
DLOC = NH * DK   # 512 local out-features for q/k/v
THETA = 10000.0

B, S, D, H = 4, 2048, 1024, 16
N_CORES = 8

KC = D // P      # 8 contraction chunks
SJ = S // SQ     # 4 q-chunks
STJ = SQ // P    # 4 s-tiles per q-chunk
ST = S // P      # 16 s-tiles total


def build_attention_program(DIN=D, DOUT=D, SEQ=S, all_reduce=True, groups=None, reps=1,
                            debug=False):
    """One SPMD Bass program. Per-core DRAM I/O:
      xt    [DIN, SEQ]   bf16  x[b].T
      wqt   [DIN, DLOC]  bf16  wq rows (perm: per-half E-block/O-block) transposed
      wkt   [DIN, DLOC]  bf16  likewise
      wvt   [DIN, DLOC]  bf16  wv rows (natural) transposed
      wot   [DLOC, DOUT] bf16  wo cols (natural) transposed
      cos4  [P, SEQ]     f32   cos table, 4x stacked [32, SEQ]
      sin4  [P, SEQ]     f32
      tri   [P, P]       bf16  upper-tri keep mask (c >= r)
      ident [P, P]       bf16  identity for PE transpose
      outp  [SJ, DOUT, SQ] f32 out^T, j-chunked
    """
    nc = bacc.Bacc(
        "TRN2",
        target_bir_lowering=False,
        debug=False,
        num_devices=(len(groups) * len(groups[0]) if groups else 1),
    )
    xt = nc.declare_dram_parameter("xt", [DIN, SEQ], BF16, isOutput=False)
    wqt = nc.declare_dram_parameter("wqt", [DIN, DLOC], BF16, isOutput=False)
    wkt = nc.declare_dram_parameter("wkt", [DIN, DLOC], BF16, isOutput=False)
    wvt = nc.declare_dram_parameter("wvt", [DIN, DLOC], BF16, isOutput=False)
    wot = nc.declare_dram_parameter("wot", [DLOC, DOUT], BF16, isOutput=False)
    cos4 = nc.declare_dram_parameter("cos4", [P, SEQ], F32, isOutput=False)
    sin4 = nc.declare_dram_parameter("sin4", [P, SEQ], F32, isOutput=False)
    tri = nc.declare_dram_parameter("tri", [P, P], BF16, isOutput=False)
    ident = nc.declare_dram_parameter("ident", [P, P], BF16, isOutput=False)
    outp = nc.declare_dram_parameter("outp", [SJ, DOUT, SQ], F32, isOutput=True)
    if debug:
        dbg_qE = nc.declare_dram_parameter("dbg_qE", [2, P, SQ], BF16, isOutput=True)
        dbg_kE = nc.declare_dram_parameter("dbg_kE", [2, P, SEQ], BF16, isOutput=True)
        dbg_qO = nc.declare_dram_parameter("dbg_qO", [2, P, SQ], BF16, isOutput=True)
        dbg_kO = nc.declare_dram_parameter("dbg_kO", [2, P, SEQ], BF16, isOutput=True)
        dbg_v = nc.declare_dram_parameter("dbg_v", [P, ST, NH, DK + 1], BF16, isOutput=True)
        dbg_ot = nc.declare_dram_parameter("dbg_ot", [P, NH * STJ * DK], BF16, isOutput=True)
        dbg_otT = nc.declare_dram_parameter("dbg_otT", [P, STJ, SQ], BF16, isOutput=True)

    from contextlib import ExitStack

    # PV psum packing: slot (head-in-half, qtile) -> (bank, idx), 6 per bank
    def pv_slice(ps, hd, qt):
        slot = hd * STJ + qt
        b, i = slot // 6, slot % 6
        return ps[:, b, i * (DK + 1) : (i + 1) * (DK + 1)]

    with tile.TileContext(nc) as tc, ExitStack() as ctx:
        ctx.enter_context(nc.allow_low_precision(reason="bf16 matmuls"))
        consts = ctx.enter_context(tc.tile_pool(name="consts", bufs=1))
        tabs = ctx.enter_context(tc.tile_pool(name="tabs", bufs=1))
        xload = ctx.enter_context(tc.tile_pool(name="xload", bufs=2))
        qk_pool = ctx.enter_context(tc.tile_pool(name="qk", bufs=1))
        v_pool = ctx.enter_context(tc.tile_pool(name="vp", bufs=1))
        ot_pool = ctx.enter_context(tc.tile_pool(name="ot", bufs=1))
        otT_pool = ctx.enter_context(tc.tile_pool(name="otT", bufs=1))
        tmp_pool = ctx.enter_context(tc.tile_pool(name="tmp", bufs=2))
        pt_pool = ctx.enter_context(tc.tile_pool(name="pt", bufs=32))
        ob_pool = ctx.enter_context(tc.tile_pool(name="ob", bufs=2))
        dram_pool = ctx.enter_context(tc.tile_pool(name="dram", bufs=2, space="DRAM"))
        psS = ctx.enter_context(tc.tile_pool(name="psS", bufs=2, space="PSUM"))
        psV = ctx.enter_context(tc.tile_pool(name="psV", bufs=1, space="PSUM"))
        for _rep in range(reps):
            tri_sb = consts.tile([P, P], BF16, tag="tri")
            nc.sync.dma_start(tri_sb[:], tri[:])
            id_sb = consts.tile([P, P], BF16, tag="ident")
            nc.sync.dma_start(id_sb[:], ident[:])
            wq_sb = consts.tile([P, KC, DLOC], BF16, tag="wq")
            nc.sync.dma_start(wq_sb[:], wqt.rearrange("(k p) c -> p k c", p=P))
            wk_sb = consts.tile([P, KC, DLOC], BF16, tag="wk")
            nc.sync.dma_start(wk_sb[:], wkt.rearrange("(k p) c -> p k c", p=P))
            wv_sb = consts.tile([P, KC, DLOC], BF16, tag="wv")
            nc.sync.dma_start(wv_sb[:], wvt.rearrange("(k p) c -> p k c", p=P))
            wo_sb = consts.tile([P, DLOC // P, DOUT], BF16, tag="wo")
            nc.sync.dma_start(wo_sb[:], wot.rearrange("(k p) c -> p k c", p=P))
            cos_sb = tabs.tile([P, SEQ], F32, tag="cos")
            nc.sync.dma_start(cos_sb[:], cos4[:])
            sin_sb = tabs.tile([P, SEQ], F32, tag="sin")
            nc.sync.dma_start(sin_sb[:], sin4[:])

            # rotated k in [dout, s] layout, per half: E block (4 heads x 32
            # even dims) and O block (odd dims). q is per-j (ring 2).
            kE = [qk_pool.tile([P, SEQ], BF16, tag=f"kE{g}", name=f"kE{g}") for g in range(2)]
            kO = [qk_pool.tile([P, SEQ], BF16, tag=f"kO{g}", name=f"kO{g}") for g in range(2)]
            # v natural [s, dv]: per s-tile, per head: 64 dims + ones col
            v_sb = v_pool.tile([P, ST, NH, DK + 1], BF16, tag="v")
            nc.vector.memset(v_sb[:, :, :, DK : DK + 1], 1.0)
            # attention out per j, head-major flat: col (hd*4+qt)*64+d
            ot_sb = ot_pool.tile([P, NH * STJ * DK], BF16, tag="ot")
            otr = ot_sb.rearrange("p (h q d) -> p h q d", q=STJ, d=DK)
            otT_sb = otT_pool.tile([P, STJ, SQ], BF16, tag="otT")

            def emit_qkv(j):
                """Projections + RoPE for chunk j. Returns (qE, qO) per-j tiles."""
                js = slice(j * SQ, (j + 1) * SQ)
                xt_sb = xload.tile([P, KC, SQ], BF16, tag="xt", name=f"xt{j}")
                nc.sync.dma_start(
                    xt_sb[:], xt[:, js].rearrange("(k p) s -> p k s", p=P)
                )
                qE = [qk_pool.tile([P, SQ], BF16, tag=f"qE{g}", name=f"qE{g}_{j}",
                                   bufs=2) for g in range(2)]
                qO = [qk_pool.tile([P, SQ], BF16, tag=f"qO{g}", name=f"qO{g}_{j}",
                                   bufs=2) for g in range(2)]
                for tname, wsb in (("q", wq_sb), ("k", wk_sb)):
                    for g in range(2):
                        ps = psS.tile([P, 2, SQ], F32, tag="sc", name=f"ps_{tname}{g}")
                        for eo in range(2):
                            c0 = g * 256 + eo * P
                            for kk in range(KC):
                                nc.tensor.matmul(
                                    ps[:, eo, :],
                                    lhsT=(wsb[:, kk, c0 : c0 + P]),
                                    rhs=(xt_sb[:, kk, :]),
                                    start=(kk == 0),
                                    stop=(kk == KC - 1),
                                )
                        # RoPE: yE = cos*E - sin*O ; yO = sin*E + cos*O
                        # psum-reading mults on DVE; SBUF-only combines on Pool
                        if tname == "q":
                            dE, dO = qE[g][:], qO[g][:]
                        else:
                            dE, dO = kE[g][:, js], kO[g][:, js]
                        t1 = tmp_pool.tile([P, SQ], F32, tag="t1")
                        nc.vector.tensor_tensor(t1[:], cos_sb[:, js], ps[:, 0, :], ALU.mult)
                        t2 = tmp_pool.tile([P, SQ], F32, tag="t2")
                        nc.vector.tensor_tensor(t2[:], sin_sb[:, js], ps[:, 1, :], ALU.mult)
                        nc.gpsimd.tensor_tensor(dE, t1[:], t2[:], ALU.subtract)
                        t3 = tmp_pool.tile([P, SQ], F32, tag="t1")
                        nc.vector.tensor_tensor(t3[:], sin_sb[:, js], ps[:, 0, :], ALU.mult)
                        t4 = tmp_pool.tile([P, SQ], F32, tag="t2")
                        nc.vector.tensor_tensor(t4[:], cos_sb[:, js], ps[:, 1, :], ALU.mult)
                        nc.gpsimd.tensor_tensor(dO, t3[:], t4[:], ALU.add)

                # v projection: per s-tile pair [s 128, dv 512]
                for u in range(2):
                    ps = psS.tile([P, 2, SQ], F32, tag="sc", name=f"ps_v{u}")
                    for i in range(2):
                        st = 2 * u + i
                        for kk in range(KC):
                            nc.tensor.matmul(
                                ps[:, i, :],
                                lhsT=(xt_sb[:, kk, st * P : (st + 1) * P]),
                                rhs=(wv_sb[:, kk, :]),
                                start=(kk == 0),
                                stop=(kk == KC - 1),
                            )
                    nc.vector.tensor_copy(
                        v_sb[:, j * STJ + 2 * u : j * STJ + 2 * u + 2, :, 0:DK],
                        ps.rearrange("p i (h d) -> p i h d", h=NH),
                    )
                return qE, qO

            def emit_scores(j, half, qE, qO, zip_work=None):
                """Phase A: scores + exp + mask for all causal tiles of
                (half, j). Returns pt tiles keyed (t, pair). Phase A is
                Act-limited; `zip_work` (a list of emitter callables of
                PE-side work with ready deps) is drained evenly between
                tiles to fill the PE gaps."""
                ntile = (j + 1) * STJ
                zw = list(zip_work or [])
                pts = {}
                for t in range(ntile):
                    ts_ = slice(t * P, (t + 1) * P)
                    tq = t - j * STJ  # >=0 on diagonal wedge
                    start_c = max(tq, 0) * P
                    for p in range(2):  # head pairs in this half
                        ssc = psS.tile([P, 2, SQ], F32, tag="sc", name=f"ssc{p}")
                        for hh in range(2):
                            hd = 2 * p + hh
                            bp = hd * 32
                            nc.tensor.matmul(
                                ssc[:, hh, start_c:SQ],
                                lhsT=(kE[half][bp : bp + 32, ts_]),
                                rhs=(qE[half][bp : bp + 32, start_c:SQ]),
                                start=True,
                                stop=False,
                                tile_position=(bp, 0),
                            )
                            nc.tensor.matmul(
                                ssc[:, hh, start_c:SQ],
                                lhsT=(kO[half][bp : bp + 32, ts_]),
                                rhs=(qO[half][bp : bp + 32, start_c:SQ]),
                                start=False,
                                stop=True,
                                tile_position=(bp, 0),
                            )
                        pt = pt_pool.tile([P, 2, SQ], BF16, tag="pt",
                                          name=f"pt{t}_{p}")
                        nc.scalar.activation(
                            pt[:, :, start_c:SQ], ssc[:, :, start_c:SQ],
                            AF.Exp, scale=0.125,
                        )
                        if tq >= 0:
                            # diagonal block: zero where q < sk (on Pool —
                            # keeps DVE free to serve psum-ring readers)
                            nc.gpsimd.tensor_tensor(
                                pt[:, :, start_c : start_c + P],
                                pt[:, :, start_c : start_c + P],
                                tri_sb[:, None, :].broadcast_to((P, 2, P)),
                                ALU.mult,
                            )
                        pts[(t, p)] = pt
                    # drain a proportional share of the zipped PE work
                    nchunk = max(1, len(zw) // (ntile - t)) if zw else 0
                    for _ in range(nchunk):
                        if zw:
                            zw.pop(0)()
                for em in zw:
                    em()
                return pts

            def pv_emitters(j, half, pts):
                """Phase B: PV chains, qt-outer; one psum bank per head
                (a start=True resets its whole 2KB bank per partition lane,
                so banks hold exactly one live accumulation chain). Batched
                norm per qt. Returned as emitter chunks for zipping."""
                pv_box = []

                def em_chain(qt, hd):
                    if not pv_box:
                        pv_box.append(
                            psV.tile([P, STJ, SQ], F32, tag="pv", name=f"pv{half}")
                        )
                    pv = pv_box[0]
                    last = j * STJ + qt
                    for t in range(last + 1):
                        nc.tensor.matmul(
                            pv[:, hd, 0 : DK + 1],
                            lhsT=(pts[(t, hd // 2)][:, hd % 2, qt * P : (qt + 1) * P]),
                            rhs=(v_sb[:, t, half * 4 + hd, :]),
                            start=(t == 0),
                            stop=(t == last),
                        )

                def em_norm(qt):
                    pv = pv_box[0]
                    rden = tmp_pool.tile([P, 4], F32, tag="rden")
                    nc.vector.reciprocal(rden[:], pv[:, :, DK])
                    nc.vector.tensor_tensor(
                        otr[:, half * 4 : half * 4 + 4, qt, :],
                        pv[:, :, 0:DK],
                        rden[:, :, None].broadcast_to((P, 4, DK)),
                        ALU.mult,
                    )

                ems = []
                for qt in range(STJ):
                    for hd in range(4):
                        ems.append(lambda qt=qt, hd=hd: em_chain(qt, hd))
                    ems.append(lambda qt=qt: em_norm(qt))
                return ems

            def emit_transpose(j, qt):
                """ot [q, dloc] -> otT [dloc, q] for one qtile: an 8-matmul
                accumulation chain (regular matmul vs identity -> f32 psum,
                eight 512B outputs disjoint within one 2KB bank)."""
                tp = psS.tile([P, STJ, P], F32, tag="sc", name=f"tp{qt}")
                for ic in range(STJ):
                    for u in range(2):
                        hd = 2 * ic + u
                        c0 = (hd * STJ + qt) * DK
                        # per-u chains: psum has_written bits are per
                        # partition lane, so each 64-partition strip
                        # needs its own start to clear stale state
                        nc.tensor.matmul(
                            tp[u * DK : (u + 1) * DK, ic, :],
                            lhsT=ot_sb[:, c0 : c0 + DK],
                            rhs=id_sb[:],
                            start=(ic == 0),
                            stop=(ic == STJ - 1),
                        )
                nc.vector.tensor_copy(otT_sb[:, :, qt * P : (qt + 1) * P], tp[:])

            def transp_oproj_emitters(j):
                """Transposes, output projection, and AllReduce for chunk j,
                as emitter chunks (zipped into the next j's Phase A)."""
                ems = [lambda qt=qt: emit_transpose(j, qt) for qt in range(STJ)]
                box = []

                def em_oproj(sblk):
                    if not box:
                        box.append(dram_pool.tile([DOUT, SQ], F32, tag="opart",
                                                  name=f"op_dram{j}"))
                    op_dram = box[0]
                    ps = psS.tile([P, 2, SQ], F32, tag="sc", name=f"op{sblk}")
                    for u in range(2):
                        dc = 2 * sblk + u
                        for ic in range(STJ):
                            nc.tensor.matmul(
                                ps[:, u, :],
                                lhsT=(wo_sb[:, ic, dc * P : (dc + 1) * P]),
                                rhs=(otT_sb[:, ic, :]),
                                start=(ic == 0),
                                stop=(ic == STJ - 1),
                            )
                    ob = ob_pool.tile([P, 2, SQ], F32, tag="ob")
                    nc.vector.tensor_copy(ob[:], ps[:])
                    for u in range(2):
                        dc = 2 * sblk + u
                        nc.sync.dma_start(
                            op_dram[dc * P : (dc + 1) * P, :], ob[:, u, :]
                        )

                def em_ar():
                    op_dram = box[0]
                    if all_reduce:
                        ar_dram = dram_pool.tile([DOUT, SQ], F32, tag="arout",
                                                 name=f"ar_dram{j}")
                        nc.gpsimd.collective_compute(
                            "AllReduce",
                            ALU.add,
                            replica_groups=groups,
                            ins=[op_dram.opt()],
                            outs=[ar_dram.opt()],
                        )
                        nc.sync.dma_start(outp[j], ar_dram[:])
                    else:
                        nc.sync.dma_start(outp[j], op_dram[:])

                ems += [lambda s=sblk: em_oproj(s) for sblk in range(DOUT // P // 2)]
                ems.append(em_ar)
                return ems

            qn = emit_qkv(0)
            pending = []
            for j in range(SJ):
                pts0 = emit_scores(j, 0, *qn, zip_work=pending)
                for em in pv_emitters(j, 0, pts0):
                    em()
                pts1 = emit_scores(j, 1, *qn)
                if j + 1 < SJ:
                    qn = emit_qkv(j + 1)
                for em in pv_emitters(j, 1, pts1):
                    em()
                pending = transp_oproj_emitters(j)
            for em in pending:
                em()
            if debug:
                nc.sync.dma_start(dbg_ot[:], ot_sb[:])
                nc.sync.dma_start(dbg_otT[:], otT_sb[:])

            if debug:
                for g in range(2):
                    nc.sync.dma_start(dbg_qE[g], qn[0][g][:])
                    nc.sync.dma_start(dbg_kE[g], kE[g][:])
                    nc.sync.dma_start(dbg_qO[g], qn[1][g][:])
                    nc.sync.dma_start(dbg_kO[g], kO[g][:])
                nc.sync.dma_start(dbg_v[:], v_sb[:])

    nc.finalize()
    return nc


def make_perms():
    """perm (q/k): per half, E-block then O-block across the half's 4 heads."""
    perm = []
    for half in range(2):
        for par in range(2):  # 0=E, 1=O
            for h in range(4 * half, 4 * half + 4):
                for i in range(32):
                    perm.append(h * DK + 2 * i + par)
    return np.array(perm)


def make_tables(token_positions, SEQ):
    pos = np.asarray(token_positions).astype(np.float32)
    inv_freq = (1.0 / (THETA ** (np.arange(0, DK, 2, dtype=np.float32) / DK))).astype(
        np.float32
    )
    freqs = pos[:, None] * inv_freq[None, :]  # [S, 32]
    cosT = np.cos(freqs).T.astype(np.float32)  # [32, S]
    sinT = np.sin(freqs).T.astype(np.float32)
    return (
        np.ascontiguousarray(np.tile(cosT, (4, 1))),
        np.ascontiguousarray(np.tile(sinT, (4, 1))),
    )


def shard_inputs(x, token_positions, wq, wk, wv, wo):
    """Build the 8 per-core input maps."""
    bf16 = mybir.dt.np(BF16)
    perm = make_perms()
    cos4, sin4 = make_tables(token_positions, x.shape[1])
    tri = np.triu(np.ones((P, P), dtype=np.float32)).astype(bf16)
    ident = np.eye(P, dtype=np.float32).astype(bf16)
    in_maps = []
    for c in range(N_CORES):
        b, hg = c // 2, c % 2
        rows = hg * DLOC
        gperm = perm + rows
        rsl = slice(rows, rows + DLOC)
        in_maps.append(
            {
                "xt": np.ascontiguousarray(x[b].T).astype(bf16),
                "wqt": np.ascontiguousarray(wq[gperm, :].T.astype(bf16)),
                "wkt": np.ascontiguousarray(wk[gperm, :].T.astype(bf16)),
                "wvt": np.ascontiguousarray(wv[rsl, :].T.astype(bf16)),
                "wot": np.ascontiguousarray(wo[:, rsl].T.astype(bf16)),
                "cos4": cos4,
                "sin4": sin4,
                "tri": tri,
                "ident": ident,
            }
        )
    return in_maps


def gather_output(res):
    out = np.empty((B, S, D), dtype=np.float32)
    for b in range(B):
        chunks = res.results[2 * b]["outp"]  # [SJ, D, SQ]
        outT = np.concatenate(list(chunks), axis=1)  # [D, S]
        out[b] = outT.T
    return out


_NC_CACHE = {}


def kernel(x, token_positions, wq, wk, wv, wo, trace=False):
    x = np.asarray(x, dtype=np.float32)
    wq = np.asarray(wq, dtype=np.float32)
    wk = np.asarray(wk, dtype=np.float32)
    wv = np.asarray(wv, dtype=np.float32)
    wo = np.asarray(wo, dtype=np.float32)

    key = "full"
    if key not in _NC_CACHE:
        _NC_CACHE[key] = build_attention_program(
            DIN=D,
            DOUT=D,
            SEQ=S,
            all_reduce=True,
            groups=[[0, 1], [2, 3], [4, 5], [6, 7]],
        )
    nc = _NC_CACHE[key]

    in_maps = shard_inputs(x, token_positions, wq, wk, wv, wo)
    res = run_bass_kernel_spmd(nc, in_maps, list(range(N_CORES)), trace=trace)
    out = gather_output(res)
    if trace:
        return out, res
    return out
